# revision 2
# baseline (speedup 1.0000x reference)
"""CrossCosineEmbeddingLoss kernel for 8 trn2 NeuronCores (v4).

loss = mean over all (i,j) of: 1 - cos(x_i, y_j) if i==j else relu(cos(x_i, y_j))
     = [ sum_ij relu(sim) + sum_i (1 - sim_ii - relu(sim_ii)) ] / n^2

Sharding (4x2 grid): core c=(a,b): x rows [2048a, 2048a+2048), y rows
[4096b, 4096b+4096).  Each core computes 32 j-tiles x 2048 i-cols of sims.
Every core also computes the diag terms vs its own x rows; host counts
those only from b==0 cores.

Per-core pipeline:
  - x: load f32 -> DVE sumsq/rsqrt -> STT normalize+cast bf16 -> DMA-XBAR
    transpose -> xhatT bf16 [128d, 16t, 128i]
  - y (8 groups of 4 tiles): load f32 -> GpSimd cast bf16 -> GpSimd square
    -> DVE seg-reduce -> rny ; DMA-XBAR transpose -> yT bf16 [128d,32t,128j]
  - main loop (32 j-tiles): 4x bf16 matmul N=512 -> PSUM [128,2048] f32;
    drain alternating ACT (relu(rny*z) accum) / DVE (STT max0*rny accum)
  - diag: GpSimd products/squares, small DVE/ACT ops
Host combines [128,2] per-core partials.
"""

import numpy as np

import concourse.bacc as bacc
import concourse.bass as bass
import concourse.tile as tile
from concourse import mybir
from concourse.bass_utils import run_bass_kernel_spmd

N, D = 8192, 128
NCORES = 8
GA, GB = 4, 2             # grid: i-blocks x j-blocks
XS = N // GA              # 2048 x rows per core
YS = N // GB              # 4096 y rows per core
XT = XS // 128            # 16 x-tiles
JT = YS // 128            # 32 j-tiles
XC = 4                    # x processed in 4 chunks of 4 tiles
YG = 8                    # y groups
GT = JT // YG             # 4 j-tiles per group

f32 = mybir.dt.float32
bf16 = mybir.dt.bfloat16
AF = mybir.ActivationFunctionType
ALU = mybir.AluOpType
AX = mybir.AxisListType

# drain split ACT : DVE ~ 17 : 15
ACT_DRAIN = set()
_acc = 0
for _t in range(JT):
    _acc += 17
    if _acc >= 32:
        _acc -= 32
        ACT_DRAIN.add(_t)

_CACHE = {}


def _build():
    if "nc" in _CACHE:
        return _CACHE["nc"]
    nc = bacc.Bacc("TRN2", target_bir_lowering=False, debug=False,
                   num_devices=NCORES)
    xs_d = nc.dram_tensor("xs", [XS, D], f32, kind="ExternalInput")
    ys_d = nc.dram_tensor("ys", [YS, D], f32, kind="ExternalInput")
    yd_d = nc.dram_tensor("yd", [XS, D], f32, kind="ExternalInput")
    out_d = nc.dram_tensor("out", [128, 2], f32, kind="ExternalOutput")

    xv = xs_d[:].rearrange("(p t) d -> p t d", t=XT)
    yv = ys_d[:].rearrange("(p t) d -> p t d", t=JT)
    ydv = yd_d[:].rearrange("(p t) d -> p t d", t=XT)

    with tile.TileContext(nc) as tc:
        with (
            tc.tile_pool(name="singles", bufs=1) as singles,
            tc.tile_pool(name="yload", bufs=3) as yload,
            tc.tile_pool(name="scr", bufs=2) as scr,
        ):
            # persistent tiles
            xnat = singles.tile([128, XT, 128], f32)
            xbf = singles.tile([128, XT, 128], bf16)
            xhatT = singles.tile([128, XT, 128], bf16)
            yT = singles.tile([128, JT, 128], bf16)
            ydn = singles.tile([128, XT, 128], f32)
            nx2 = singles.tile([128, XT], f32)
            rnx = singles.tile([128, XT], f32)
            ny2 = singles.tile([128, JT], f32)
            rny = singles.tile([128, JT], f32)
            t1x = singles.tile([128, XT], f32)
            t2y = singles.tile([128, JT], f32)
            racc = singles.tile([128, JT], f32)
            prodd = singles.tile([128, XT, 128], bf16)
            sqd = singles.tile([128, XT, 128], bf16)
            d2 = singles.tile([128, XT], f32)
            nyd2 = singles.tile([128, XT], f32)
            rnyd = singles.tile([128, XT], f32)
            simd = singles.tile([128, XT], f32)
            relud = singles.tile([128, XT], f32)
            sd_scr = singles.tile([128, XT], f32)
            outsb = singles.tile([128, 2], f32)

            # ---- x path: 4 chunks of 4 tiles
            for c in range(XC):
                ts = slice(4 * c, 4 * c + 4)
                nc.sync.dma_start(out=xnat[:, ts, :], in_=xv[:, ts, :])
                sq = scr.tile([128, 4, 128], f32, tag="xsq")
                nc.vector.tensor_tensor(
                    out=sq[:], in0=xnat[:, ts, :], in1=xnat[:, ts, :],
                    op=ALU.mult)
                nc.vector.tensor_reduce(
                    out=nx2[:, ts], in_=sq[:], axis=AX.X, op=ALU.add)
                nc.vector.reciprocal(t1x[:, ts], nx2[:, ts])
                nc.scalar.sqrt(rnx[:, ts], t1x[:, ts])
                nc.vector.scalar_tensor_tensor(
                    out=xbf[:, ts, :], in0=xnat[:, ts, :], scalar=1.0,
                    in1=rnx[:, ts].unsqueeze(2).broadcast_to([128, 4, 128]),
                    op0=ALU.mult, op1=ALU.mult)
                for k in range(4):
                    t = 4 * c + k
                    nc.sync.dma_start(out=xhatT[:, t, :], in_=xbf[:, t, :],
                                      transpose=True)

            # ---- y path: 8 groups of 4 tiles
            for g in range(YG):
                ts = slice(GT * g, GT * (g + 1))
                ynat = yload.tile([128, GT, 128], f32, tag="ynat")
                nc.sync.dma_start(out=ynat[:], in_=yv[:, ts, :])
                ybf = yload.tile([128, GT, 128], bf16, tag="ybf")
                nc.gpsimd.tensor_copy(out=ybf[:], in_=ynat[:])
                sq = scr.tile([128, GT, 128], bf16, tag="ysq")
                nc.gpsimd.tensor_tensor(
                    out=sq[:], in0=ybf[:], in1=ybf[:], op=ALU.mult)
                nc.vector.tensor_reduce(
                    out=ny2[:, ts], in_=sq[:], axis=AX.X, op=ALU.add)
                nc.vector.reciprocal(t2y[:, ts], ny2[:, ts])
                nc.scalar.sqrt(rny[:, ts], t2y[:, ts])
                for k in range(GT):
                    t = GT * g + k
                    nc.sync.dma_start(out=yT[:, t, :], in_=ybf[:, k, :],
                                      transpose=True)

            # ---- main loop
            rhs = xhatT[:].rearrange("p a b -> p (a b)")
            with tc.tile_pool(name="mpsum", bufs=2, space="PSUM") as mpsum:
                for t in range(JT):
                    ps = mpsum.tile([128, 2048], f32, tag="mp")
                    lhsT = yT[:, t, :]
                    for q in range(4):
                        nc.tensor.matmul(ps[:, 512 * q:512 * (q + 1)], lhsT,
                                         rhs[:, 512 * q:512 * (q + 1)])
                    if t in ACT_DRAIN:
                        nc.scalar.activation(
                            ps[:], ps[:], AF.Relu, scale=rny[:, t:t + 1],
                            accum_out=racc[:, t:t + 1])
                    else:
                        nc.vector.scalar_tensor_tensor(
                            out=ps[:], in0=ps[:], scalar=0.0,
                            in1=rny[:, t:t + 1].broadcast_to([128, 2048]),
                            op0=ALU.max, op1=ALU.mult,
                            accum_out=racc[:, t:t + 1])

            # ---- diag terms (all cores; host uses b==0 cores only)
            nc.sync.dma_start(out=ydn[:], in_=ydv[:])
            nc.gpsimd.tensor_tensor(
                out=prodd[:], in0=xnat[:], in1=ydn[:], op=ALU.mult)
            nc.gpsimd.tensor_tensor(
                out=sqd[:], in0=ydn[:], in1=ydn[:], op=ALU.mult)
            nc.vector.tensor_reduce(
                out=d2[:], in_=prodd[:], axis=AX.X, op=ALU.add)
            nc.vector.tensor_reduce(
                out=nyd2[:], in_=sqd[:], axis=AX.X, op=ALU.add)
            nc.vector.reciprocal(sd_scr[:], nyd2[:])
            nc.scalar.sqrt(rnyd[:], sd_scr[:])
            # sim_ii = d2 * rnx * rnyd
            nc.vector.tensor_tensor(out=simd[:], in0=d2[:], in1=rnx[:],
                                    op=ALU.mult)
            nc.vector.tensor_tensor(out=simd[:], in0=simd[:], in1=rnyd[:],
                                    op=ALU.mult)
            nc.scalar.activation(relud[:], simd[:], AF.Relu)
            nc.vector.scalar_tensor_tensor(
                out=sd_scr[:], in0=simd[:], scalar=1.0, in1=relud[:],
                op0=ALU.mult, op1=ALU.add, accum_out=outsb[:, 1:2])

            # ---- tail
            nc.vector.tensor_reduce(out=outsb[:, 0:1], in_=racc[:],
                                    axis=AX.X, op=ALU.add)
            nc.sync.dma_start(out=out_d[:], in_=outsb[:])

    nc.compile()
    _CACHE["nc"] = nc
    return nc


def _in_maps(x, y):
    maps = []
    for c in range(NCORES):
        a, b = c // GB, c % GB
        maps.append({
            "xs": np.ascontiguousarray(x[XS * a:XS * (a + 1)]),
            "ys": np.ascontiguousarray(y[YS * b:YS * (b + 1)]),
            "yd": np.ascontiguousarray(y[XS * a:XS * (a + 1)]),
        })
    return maps


def _combine(results):
    total = 0.0
    for c in range(NCORES):
        o = results[c]["out"].astype(np.float64)
        total += o[:, 0].sum()
        if c % GB == 0:
            total += XS - o[:, 1].sum()
    return np.float32(total / (float(N) * float(N)))


def _run(x, y, trace=False):
    nc = _build()
    res = run_bass_kernel_spmd(nc, _in_maps(x, y), list(range(NCORES)),
                               trace=trace)
    return _combine(res.results), res


def kernel(x, y):
    x = np.asarray(x, dtype=np.float32)
    y = np.asarray(y, dtype=np.float32)
    loss, _ = _run(x, y, trace=False)
    return loss


# revision 3
# speedup vs baseline: 1.2889x; 1.2889x over previous
"""CrossCosineEmbeddingLoss kernel for 8 trn2 NeuronCores (v5).

loss = mean over all (i,j) of: 1 - cos(x_i, y_j) if i==j else relu(cos(x_i, y_j))
     = [ sum_ij relu(sim) + sum_i (1 - sim_ii - relu(sim_ii)) ] / n^2

Sharding (4x2 grid): core c=(a,b): x rows [2048a, 2048a+2048), y rows
[4096b, 4096b+4096).  Each core computes 32 j-tiles x 2048 i-cols of sims.
Every core computes diag terms vs its own x rows; host counts them only
from b==0 cores.

Per-core pipeline:
  - x (4 chunks of 4 tiles, tile-major layout [i_lo, c, d]): DVE sumsq ->
    rsqrt -> ACT per-tile scale+cast bf16 -> batched DMA-XBAR transpose ->
    xhatT bf16 [128d, 16t, 128i]
  - y (4 groups of 8 tiles, tile-major [j_lo, a, d]): GpSimd cast bf16 +
    GpSimd squares -> DVE seg-reduce -> rny ; one DMA-XBAR transpose per
    group -> yT bf16 [128d, 32t, 128j]
  - main (32 j-tiles): 4x bf16 matmul N=512 -> PSUM [128,2048] f32; drain
    alternating ACT relu(rny*z)+accum / DVE STT max0*rny+accum (relu
    commutes with the positive per-row scale rny)
  - diag: GpSimd products/squares + small DVE/ACT ops
Host combines [128,2] per-core partials.
"""

import numpy as np

import concourse.bacc as bacc
import concourse.bass as bass
import concourse.tile as tile
from concourse import mybir
from concourse.bass_utils import run_bass_kernel_spmd

N, D = 8192, 128
NCORES = 8
GA, GB = 4, 2             # grid: i-blocks x j-blocks
XS = N // GA              # 2048 x rows per core
YS = N // GB              # 4096 y rows per core
XT = XS // 128            # 16 x-tiles
JT = YS // 128            # 32 j-tiles
XC = 4                    # x chunks
XCT = XT // XC            # 4 tiles per x chunk
YG = 4                    # y groups
GT = JT // YG             # 8 j-tiles per group

f32 = mybir.dt.float32
bf16 = mybir.dt.bfloat16
AF = mybir.ActivationFunctionType
ALU = mybir.AluOpType
AX = mybir.AxisListType

# drain split ACT : DVE ~ 17 : 15 (measured 1.92 vs 2.28 us/tile)
ACT_DRAIN = set()
_acc = 0
for _t in range(JT):
    _acc += 17
    if _acc >= 32:
        _acc -= 32
        ACT_DRAIN.add(_t)

_CACHE = {}


def _build():
    if "nc" in _CACHE:
        return _CACHE["nc"]
    nc = bacc.Bacc("TRN2", target_bir_lowering=False, debug=False,
                   num_devices=NCORES)
    xs_d = nc.dram_tensor("xs", [XS, D], f32, kind="ExternalInput")
    ys_d = nc.dram_tensor("ys", [YS, D], f32, kind="ExternalInput")
    yd_d = nc.dram_tensor("yd", [XS, D], f32, kind="ExternalInput")
    out_d = nc.dram_tensor("out", [128, 2], f32, kind="ExternalOutput")

    # tile-major views: [p, tile, d] with row = 128*tile + p
    xv = xs_d[:].rearrange("(c p) d -> p c d", c=XT)
    yv = ys_d[:].rearrange("(a p) d -> p a d", a=JT)
    ydv = yd_d[:].rearrange("(c p) d -> p c d", c=XT)

    with tile.TileContext(nc) as tc:
        with (
            tc.tile_pool(name="singles", bufs=1) as singles,
            tc.tile_pool(name="yload", bufs=2) as yload,
            tc.tile_pool(name="scr", bufs=2) as scr,
        ):
            xnat = singles.tile([128, XT, 128], f32)
            xbf = singles.tile([128, XT, 128], bf16)
            xhatT = singles.tile([128, XT, 128], bf16)
            yT = singles.tile([128, JT, 128], bf16)
            ybfs = [singles.tile([128, GT, 128], bf16, name=f"ybf{g}")
                    for g in range(YG)]
            ydn = singles.tile([128, XT, 128], f32)
            nx2 = singles.tile([128, XT], f32)
            rnx = singles.tile([128, XT], f32)
            ny2 = singles.tile([128, JT], f32)
            rny = singles.tile([128, JT], f32)
            t1x = singles.tile([128, XT], f32)
            t2y = singles.tile([128, JT], f32)
            racc = singles.tile([128, JT], f32)
            prodd = singles.tile([128, XT, 128], bf16)
            sqd = singles.tile([128, XT, 128], bf16)
            d2 = singles.tile([128, XT], f32)
            nyd2 = singles.tile([128, XT], f32)
            rnyd = singles.tile([128, XT], f32)
            simd = singles.tile([128, XT], f32)
            relud = singles.tile([128, XT], f32)
            sd_scr = singles.tile([128, XT], f32)
            outsb = singles.tile([128, 2], f32)

            # ---- y group 0 first (unblocks first MMs + first drains)
            def y_group(g):
                ts = slice(GT * g, GT * (g + 1))
                ynat = yload.tile([128, GT, 128], f32, tag="ynat")
                nc.sync.dma_start(out=ynat[:], in_=yv[:, ts, :])
                ybf = ybfs[g]
                nc.gpsimd.tensor_copy(out=ybf[:], in_=ynat[:])
                nc.sync.dma_start(out=yT[:, ts, :], in_=ybf[:],
                                  transpose=True)
                sq = scr.tile([128, GT, 128], bf16, tag="ysq")
                nc.gpsimd.tensor_tensor(
                    out=sq[:], in0=ybf[:], in1=ybf[:], op=ALU.mult)
                nc.vector.tensor_reduce(
                    out=ny2[:, ts], in_=sq[:], axis=AX.X, op=ALU.add)
                nc.vector.reciprocal(t2y[:, ts], ny2[:, ts])
                nc.scalar.sqrt(rny[:, ts], t2y[:, ts])

            def x_chunk(c):
                ts = slice(XCT * c, XCT * (c + 1))
                nc.sync.dma_start(out=xnat[:, ts, :], in_=xv[:, ts, :])
                sq = scr.tile([128, XCT, 128], f32, tag="xsq")
                nc.vector.tensor_tensor(
                    out=sq[:], in0=xnat[:, ts, :], in1=xnat[:, ts, :],
                    op=ALU.mult)
                nc.vector.tensor_reduce(
                    out=nx2[:, ts], in_=sq[:], axis=AX.X, op=ALU.add)
                nc.vector.reciprocal(t1x[:, ts], nx2[:, ts])
                nc.scalar.sqrt(rnx[:, ts], t1x[:, ts])
                for k in range(XCT):
                    t = XCT * c + k
                    nc.scalar.activation(
                        xbf[:, t, :], xnat[:, t, :], AF.Copy,
                        scale=rnx[:, t:t + 1])
                nc.scalar.dma_start(out=xhatT[:, ts, :], in_=xbf[:, ts, :],
                                    transpose=True)

            y_group(0)
            for c in range(XC):
                x_chunk(c)
            for g in range(1, YG):
                y_group(g)

            # ---- diag inputs (gpsimd heavy part early; small ops later)
            nc.sync.dma_start(out=ydn[:], in_=ydv[:])
            nc.gpsimd.tensor_tensor(
                out=prodd[:], in0=xnat[:], in1=ydn[:], op=ALU.mult)
            nc.gpsimd.tensor_tensor(
                out=sqd[:], in0=ydn[:], in1=ydn[:], op=ALU.mult)

            # ---- main loop
            rhs = xhatT[:].rearrange("p a b -> p (a b)")
            with tc.tile_pool(name="mpsum", bufs=2, space="PSUM") as mpsum:
                for t in range(JT):
                    ps = mpsum.tile([128, 2048], f32, tag="mp")
                    lhsT = yT[:, t, :]
                    for q in range(4):
                        nc.tensor.matmul(ps[:, 512 * q:512 * (q + 1)], lhsT,
                                         rhs[:, 512 * q:512 * (q + 1)])
                    if t in ACT_DRAIN:
                        nc.scalar.activation(
                            ps[:], ps[:], AF.Relu, scale=rny[:, t:t + 1],
                            accum_out=racc[:, t:t + 1])
                    else:
                        nc.vector.scalar_tensor_tensor(
                            out=ps[:], in0=ps[:], scalar=0.0,
                            in1=rny[:, t:t + 1].broadcast_to([128, 2048]),
                            op0=ALU.max, op1=ALU.mult,
                            accum_out=racc[:, t:t + 1])

            # ---- diag small ops
            nc.vector.tensor_reduce(
                out=d2[:], in_=prodd[:], axis=AX.X, op=ALU.add)
            nc.vector.tensor_reduce(
                out=nyd2[:], in_=sqd[:], axis=AX.X, op=ALU.add)
            nc.vector.reciprocal(sd_scr[:], nyd2[:])
            nc.scalar.sqrt(rnyd[:], sd_scr[:])
            nc.vector.tensor_tensor(out=simd[:], in0=d2[:], in1=rnx[:],
                                    op=ALU.mult)
            nc.vector.tensor_tensor(out=simd[:], in0=simd[:], in1=rnyd[:],
                                    op=ALU.mult)
            nc.scalar.activation(relud[:], simd[:], AF.Relu)
            nc.vector.scalar_tensor_tensor(
                out=sd_scr[:], in0=simd[:], scalar=1.0, in1=relud[:],
                op0=ALU.mult, op1=ALU.add, accum_out=outsb[:, 1:2])

            # ---- tail
            nc.vector.tensor_reduce(out=outsb[:, 0:1], in_=racc[:],
                                    axis=AX.X, op=ALU.add)
            nc.sync.dma_start(out=out_d[:], in_=outsb[:])

    nc.compile()
    _CACHE["nc"] = nc
    return nc


def _in_maps(x, y):
    maps = []
    for c in range(NCORES):
        a, b = c // GB, c % GB
        maps.append({
            "xs": np.ascontiguousarray(x[XS * a:XS * (a + 1)]),
            "ys": np.ascontiguousarray(y[YS * b:YS * (b + 1)]),
            "yd": np.ascontiguousarray(y[XS * a:XS * (a + 1)]),
        })
    return maps


def _combine(results):
    total = 0.0
    for c in range(NCORES):
        o = results[c]["out"].astype(np.float64)
        total += o[:, 0].sum()
        if c % GB == 0:
            total += XS - o[:, 1].sum()
    return np.float32(total / (float(N) * float(N)))


def _run(x, y, trace=False):
    nc = _build()
    res = run_bass_kernel_spmd(nc, _in_maps(x, y), list(range(NCORES)),
                               trace=trace)
    return _combine(res.results), res


def kernel(x, y):
    x = np.asarray(x, dtype=np.float32)
    y = np.asarray(y, dtype=np.float32)
    loss, _ = _run(x, y, trace=False)
    return loss
